# revision 1
# baseline (speedup 1.0000x reference)
"""Dense MLP y = x @ W.T + b on 8 TRN2 NeuronCores, data-parallel over batch.

Full inputs: x [8192, 1024] f32, W [1024, 1024] f32, b [1024] f32.
Each core computes a [1024, 1024] slice of the output.

Per-core kernel computes the transposed output
    outT[n, m] = sum_k WT[k, n] * xT[k, m] + b[n]
so the bias lands on the partition dim (n) and fuses into the PSUM
eviction as a DVE tensor_scalar add. Host pre-transposes x-shards and W
to K-major (contraction on partitions) and un-transposes the gathered
outputs; only device time counts.

Raw Bass (no TileContext: its exit drain trips "Too many sync wait
commands" in this compiler build).

Engine layout (v3, trace-driven):
  sync:   ALL load DMAs on one HWDGE queue, in exact first-use order
          ([wt_c0[k], xt_c0[k]] pairs, then wt_c1, then xt_c1). The
          aggregate DMA rate caps at ~400 GB/s no matter how many
          queues issue, and queues do NOT share bandwidth fairly, so
          one priority-ordered queue beats two racing ones.
  scalar: output stores (idle queue; stores only need a ~130 GB/s
          trickle and must not displace load descriptors).
  gpsimd: bias load (SWDGE, off the critical queues).
  tensor: four k-outer phases over 4 PSUM banks each with per-slice
          gating - each k-slice feeds 4 matmuls the moment it lands,
          so only ~7 us of compute remains after the last load byte.
  vector: PSUM->SBUF evictions with fused bias add.
All matmul operands are float32r end to end (DRAM + SBUF) - the BIR
verifier requires fp32r matmul inputs to be produced as fp32r, and
fp32r streams 4x faster than plain fp32 through the PE at moving dim
512 (1 cycle/row).
"""

import numpy as np

import concourse.bass as bass
import concourse.mybir as mybir
from concourse.bass_utils import run_bass_kernel_spmd

B, IN_F, OUT_F = 8192, 1024, 1024
N_CORES = 8
M = B // N_CORES  # batch rows per core
P = 128           # partitions
MB = 512          # moving-dim block (one PSUM bank of fp32)
KT = IN_F // P    # k tiles (8)
NT = OUT_F // P   # n tiles (8)
CB = 512          # column-block width (2KB DMA lines per partition)
NGROUPS = (M // MB) * NT  # 16 psum groups, order g = mb*NT + nt

F32 = mybir.dt.float32
F32R = mybir.dt.float32r


def build_program() -> bass.Bass:
    nc = bass.Bass()
    xT = nc.declare_dram_parameter("xT", [IN_F, M], F32R, isOutput=False)
    wT = nc.declare_dram_parameter("wT", [IN_F, OUT_F], F32R, isOutput=False)
    bias = nc.declare_dram_parameter("bias", [P, NT], F32, isOutput=False)
    outT = nc.declare_dram_parameter("outT", [OUT_F, M], F32, isOutput=True)

    import contextlib

    with contextlib.ExitStack() as ctx:
        wt_sb = [
            [ctx.enter_context(nc.sbuf_tensor(f"wt{k}_{c}", [P, CB], F32R))
             for c in range(2)]
            for k in range(KT)
        ]
        xt_sb = [
            [ctx.enter_context(nc.sbuf_tensor(f"xt{k}_{c}", [P, CB], F32R))
             for c in range(2)]
            for k in range(KT)
        ]
        ot_sb = [
            ctx.enter_context(nc.sbuf_tensor(f"ot{j}", [P, MB], F32))
            for j in range(4)
        ]
        bias_sb = ctx.enter_context(nc.sbuf_tensor("bias_sb", [P, NT], F32))
        ps = [
            ctx.enter_context(nc.psum_tensor(f"ps{b}", [P, MB], F32))
            for b in range(8)
        ]
        ld_b = ctx.enter_context(nc.semaphore("ld_b"))
        # Per (k-slice, column-block) load sems: a shared counter can't
        # prove a *specific* DMA finished (completions are unordered),
        # a single-incrementer sem can.
        ld_w = [
            [ctx.enter_context(nc.semaphore(f"ld_w{k}_{c}")) for c in range(2)]
            for k in range(KT)
        ]
        ld_x = [
            [ctx.enter_context(nc.semaphore(f"ld_x{k}_{c}")) for c in range(2)]
            for k in range(KT)
        ]
        mm = ctx.enter_context(nc.semaphore("mm"))
        ev = ctx.enter_context(nc.semaphore("ev"))
        ev_h = ctx.enter_context(nc.semaphore("ev_h"))  # last-group halves
        # Per-ot-slot store sems (same unordered-completion argument).
        st_sems = [
            ctx.enter_context(nc.semaphore(f"st{j}")) for j in range(4)
        ]

        with nc.Block(no_gpsimd_drain=True) as block:

            @block.sync
            def _(sync):
                # ALL loads on one FIFO queue in exact first-use order:
                # the DMA fabric caps at ~390-400 GB/s aggregate no
                # matter how many queues issue (two queues just split it
                # and scramble the priority order).
                for k in range(KT):
                    sync.dma_start(
                        out=wt_sb[k][0][:],
                        in_=wT[k * P:(k + 1) * P, 0:CB],
                    ).then_inc(ld_w[k][0], 16)
                    sync.dma_start(
                        out=xt_sb[k][0][:],
                        in_=xT[k * P:(k + 1) * P, 0:CB],
                    ).then_inc(ld_x[k][0], 16)
                for k in range(KT):
                    sync.dma_start(
                        out=wt_sb[k][1][:],
                        in_=wT[k * P:(k + 1) * P, CB:2 * CB],
                    ).then_inc(ld_w[k][1], 16)
                for k in range(KT):
                    sync.dma_start(
                        out=xt_sb[k][1][:],
                        in_=xT[k * P:(k + 1) * P, CB:2 * CB],
                    ).then_inc(ld_x[k][1], 16)

            @block.gpsimd
            def _(gpsimd):
                gpsimd.dma_start(out=bias_sb[:], in_=bias[:]).then_inc(ld_b, 16)

            @block.scalar
            def _(scalar):
                # Stores on the idle scalar queue (~130 GB/s trickle,
                # must not displace load descriptors). Last group is
                # split in half so the final eviction->store->drain
                # chain is shorter.
                for g in range(NGROUPS - 1):
                    mb, nt = divmod(g, NT)
                    scalar.wait_ge(ev, g + 1)
                    scalar.dma_start(
                        out=outT[nt * P:(nt + 1) * P, mb * MB:(mb + 1) * MB],
                        in_=ot_sb[g % 4][:],
                    ).then_inc(st_sems[g % 4], 16)
                for h in range(2):
                    scalar.wait_ge(ev_h, h + 1)
                    scalar.dma_start(
                        out=outT[7 * P:8 * P,
                                 MB + h * (MB // 2):MB + (h + 1) * (MB // 2)],
                        in_=ot_sb[3][:, h * (MB // 2):(h + 1) * (MB // 2)],
                    ).then_inc(st_sems[3], 16)
                for j in range(3):
                    scalar.wait_ge(st_sems[j], (NGROUPS // 4) * 16)
                scalar.wait_ge(st_sems[3], 5 * 16)

            @block.tensor
            def _(tensor):
                # Three k-outer phases over 4 PSUM banks each. Group ids
                # (= mm/ev order): P0 -> g0-3 (nt0-3, mb0, banks 0-3),
                # P1 -> g4-7 (nt4-7, mb0, banks 4-7), P2 -> g8-11
                # (nt0-3, mb1, banks 0-3). Each k-slice feeds 4 matmuls
                # as soon as it lands.
                for phase in range(3):
                    mb = phase // 2          # 0,0,1
                    cw = phase % 2           # wt column block 0,1,0
                    bank0 = cw * 4           # banks 0-3 / 4-7
                    if phase == 2:
                        tensor.wait_ge(ev, 4)   # banks 0-3 evicted (P0)
                    for k in range(KT):
                        if phase == 0:
                            tensor.wait_ge(ld_w[k][0], 16)
                            tensor.wait_ge(ld_x[k][0], 16)
                        elif phase == 1:
                            tensor.wait_ge(ld_w[k][1], 16)
                        elif phase == 2:
                            tensor.wait_ge(ld_x[k][1], 16)
                        for j in range(4):
                            inst = tensor.matmul(
                                ps[bank0 + j][:, :],
                                wt_sb[k][cw][:, j * P:(j + 1) * P],
                                xt_sb[k][mb][:, :],
                                start=(k == 0),
                                stop=(k == KT - 1),
                            )
                            if k == KT - 1:
                                inst.then_inc(mm, 1)
                # Last phase (nt4-7, mb1, banks 4-7) k-inner: group
                # completions land ~1.9us apart so evictions + stores
                # pipeline instead of bunching at the end.
                tensor.wait_ge(ev, 8)   # banks 4-7 evicted (P1)
                for g in range(12, NGROUPS):
                    nt = g - 8
                    ni = nt - 4
                    inst = None
                    for k in range(KT):
                        inst = tensor.matmul(
                            ps[4 + ni][:, :],
                            wt_sb[k][1][:, ni * P:(ni + 1) * P],
                            xt_sb[k][1][:, :],
                            start=(k == 0),
                            stop=(k == KT - 1),
                        )
                    inst.then_inc(mm, 1)

            @block.vector
            def _(vector):
                vector.wait_ge(ld_b, 16)
                for g in range(NGROUPS - 1):
                    mb, nt = divmod(g, NT)
                    vector.wait_ge(mm, g + 1)
                    if g >= 4:
                        # ot slot g%4 reused: all issued slot stores
                        # (groups g%4, g%4+4, ..., g-4) must be done
                        vector.wait_ge(st_sems[g % 4], (g // 4) * 16)
                    vector.tensor_scalar_add(
                        ot_sb[g % 4][:],
                        ps[g % 8][:, :],
                        bias_sb[:, nt:nt + 1],
                    ).then_inc(ev, 1)
                # Last group in halves: first half's store overlaps the
                # second half's eviction, shortening the critical tail.
                vector.wait_ge(mm, NGROUPS)
                vector.wait_ge(st_sems[3], 48)
                for h in range(2):
                    vector.tensor_scalar_add(
                        ot_sb[3][:, h * (MB // 2):(h + 1) * (MB // 2)],
                        ps[7][:, h * (MB // 2):(h + 1) * (MB // 2)],
                        bias_sb[:, 7:8],
                    ).then_inc(ev_h, 1)

    return nc


_PROGRAM = None


def _get_program() -> bass.Bass:
    global _PROGRAM
    if _PROGRAM is None:
        _PROGRAM = build_program()
    return _PROGRAM


def make_in_maps(x: np.ndarray, W: np.ndarray, b: np.ndarray) -> list[dict]:
    WT = np.ascontiguousarray(W.T.astype(np.float32, copy=False))
    bias = np.ascontiguousarray(
        b.astype(np.float32, copy=False).reshape(NT, P).T
    )
    in_maps = []
    for c in range(N_CORES):
        xT = np.ascontiguousarray(
            x[c * M:(c + 1) * M, :].T.astype(np.float32, copy=False)
        )
        in_maps.append({"xT": xT, "wT": WT, "bias": bias})
    return in_maps


def assemble_output(results: list[dict]) -> np.ndarray:
    out = np.empty((B, OUT_F), dtype=np.float32)
    for c in range(N_CORES):
        out[c * M:(c + 1) * M, :] = results[c]["outT"].T
    return out


def kernel(x: np.ndarray, W: np.ndarray, b: np.ndarray) -> np.ndarray:
    nc = _get_program()
    in_maps = make_in_maps(np.asarray(x), np.asarray(W), np.asarray(b))
    res = run_bass_kernel_spmd(nc, in_maps, list(range(N_CORES)))
    return assemble_output(res.results)



# revision 2
# speedup vs baseline: 1.1003x; 1.1003x over previous
"""Dense MLP y = x @ W.T + b on 8 TRN2 NeuronCores, data-parallel over batch.

Full inputs: x [8192, 1024] f32, W [1024, 1024] f32, b [1024] f32.
Each core computes a [1024, 1024] slice of the output.

Per-core kernel computes the transposed output
    outT[n, m] = sum_k WT[k, n] * xT[k, m] + b[n]
so the bias lands on the partition dim (n) and fuses into the PSUM
eviction as a DVE tensor_scalar add. Host pre-transposes x-shards and W
to K-major (contraction on partitions) and un-transposes the gathered
outputs; only device time counts.

v4: bf16 end to end (matmul inputs AND the stored output; PSUM still
accumulates f32). Measured rel err ~4e-3 vs the 2e-2 gate. Why bf16:
the v3 trace showed the kernel DMA-bound end to end - 8 MB of f32r
loads + 4 MB of f32 stores at the ~310 GB/s aggregate fabric rate
meant the last x-slice landed at ~40 us of a 54 us kernel. bf16 halves
every DMA byte (4 MB loads + 2 MB stores) while the PE still streams
1 cycle/row at moving dim 512, so the kernel flips to PE-bound:
~27.3 us of matmul + ~7.2 us fixed framework preamble + DMA spin-up.

Raw Bass (no TileContext: its exit drain trips "Too many sync wait
commands" in this compiler build).

Engine layout (trace-driven):
  sync:   ALL load DMAs on one HWDGE queue, in exact first-use order
          ([wt_c0[k], xt_c0[k]] pairs, then wt_c1, then xt_c1). The
          fabric spreads one queue across all 16 DMA engines; extra
          queues just split the same aggregate rate.
  scalar: output stores (idle queue; stores only need a trickle and
          must not displace load descriptors).
  gpsimd: bias load (SWDGE, off the critical queues).
  tensor: four k-outer phases over 4 PSUM banks each with per-slice
          gating - each k-slice feeds 4 matmuls the moment it lands.
  vector: PSUM->SBUF evictions with fused bias add (f32 psum -> bf16).
"""

import numpy as np
import ml_dtypes

import concourse.bass as bass
import concourse.mybir as mybir
from concourse.bass_utils import run_bass_kernel_spmd

B, IN_F, OUT_F = 8192, 1024, 1024
N_CORES = 8
M = B // N_CORES  # batch rows per core
P = 128           # partitions
MB = 512          # moving-dim block (one PSUM bank of fp32)
KT = IN_F // P    # k tiles (8)
NT = OUT_F // P   # n tiles (8)
CB = 512          # column-block width (1KB bf16 DMA lines per partition)
NGROUPS = (M // MB) * NT  # 16 psum groups, order g = mb*NT + nt

F32 = mybir.dt.float32
BF16 = mybir.dt.bfloat16


def build_program() -> bass.Bass:
    nc = bass.Bass()
    xT = nc.declare_dram_parameter("xT", [IN_F, M], BF16, isOutput=False)
    wT = nc.declare_dram_parameter("wT", [IN_F, OUT_F], BF16, isOutput=False)
    bias = nc.declare_dram_parameter("bias", [P, NT], F32, isOutput=False)
    outT = nc.declare_dram_parameter("outT", [OUT_F, M], BF16, isOutput=True)

    import contextlib

    with contextlib.ExitStack() as ctx:
        wt_sb = [
            [ctx.enter_context(nc.sbuf_tensor(f"wt{k}_{c}", [P, CB], BF16))
             for c in range(2)]
            for k in range(KT)
        ]
        xt_sb = [
            [ctx.enter_context(nc.sbuf_tensor(f"xt{k}_{c}", [P, CB], BF16))
             for c in range(2)]
            for k in range(KT)
        ]
        ot_sb = [
            ctx.enter_context(nc.sbuf_tensor(f"ot{j}", [P, MB], BF16))
            for j in range(4)
        ]
        bias_sb = ctx.enter_context(nc.sbuf_tensor("bias_sb", [P, NT], F32))
        ps = [
            ctx.enter_context(nc.psum_tensor(f"ps{b}", [P, MB], F32))
            for b in range(8)
        ]
        ld_b = ctx.enter_context(nc.semaphore("ld_b"))
        # Per (k-slice, column-block) load sems: a shared counter can't
        # prove a *specific* DMA finished (completions are unordered),
        # a single-incrementer sem can.
        ld_w = [
            [ctx.enter_context(nc.semaphore(f"ld_w{k}_{c}")) for c in range(2)]
            for k in range(KT)
        ]
        ld_x = [
            [ctx.enter_context(nc.semaphore(f"ld_x{k}_{c}")) for c in range(2)]
            for k in range(KT)
        ]
        mm = ctx.enter_context(nc.semaphore("mm"))
        ev = ctx.enter_context(nc.semaphore("ev"))
        ev_h = ctx.enter_context(nc.semaphore("ev_h"))  # last-group halves
        # Per-ot-slot store sems (same unordered-completion argument).
        st_sems = [
            ctx.enter_context(nc.semaphore(f"st{j}")) for j in range(4)
        ]

        with nc.Block(no_gpsimd_drain=True) as block:

            @block.sync
            def _(sync):
                # ALL loads on one FIFO queue in exact first-use order:
                # the fabric spreads one queue across all 16 DMA engines
                # at the full aggregate rate; extra queues just split it
                # and scramble the priority order.
                for k in range(KT):
                    sync.dma_start(
                        out=wt_sb[k][0][:],
                        in_=wT[k * P:(k + 1) * P, 0:CB],
                    ).then_inc(ld_w[k][0], 16)
                    sync.dma_start(
                        out=xt_sb[k][0][:],
                        in_=xT[k * P:(k + 1) * P, 0:CB],
                    ).then_inc(ld_x[k][0], 16)
                for k in range(KT):
                    sync.dma_start(
                        out=wt_sb[k][1][:],
                        in_=wT[k * P:(k + 1) * P, CB:2 * CB],
                    ).then_inc(ld_w[k][1], 16)
                for k in range(KT):
                    sync.dma_start(
                        out=xt_sb[k][1][:],
                        in_=xT[k * P:(k + 1) * P, CB:2 * CB],
                    ).then_inc(ld_x[k][1], 16)

            @block.gpsimd
            def _(gpsimd):
                gpsimd.dma_start(out=bias_sb[:], in_=bias[:]).then_inc(ld_b, 16)

            @block.scalar
            def _(scalar):
                # Stores on the idle scalar queue (trickle; must not
                # displace load descriptors). Last group is split in
                # half so the final eviction->store->drain chain is
                # shorter.
                for g in range(NGROUPS - 1):
                    mb, nt = divmod(g, NT)
                    scalar.wait_ge(ev, g + 1)
                    scalar.dma_start(
                        out=outT[nt * P:(nt + 1) * P, mb * MB:(mb + 1) * MB],
                        in_=ot_sb[g % 4][:],
                    ).then_inc(st_sems[g % 4], 16)
                for h in range(2):
                    scalar.wait_ge(ev_h, h + 1)
                    scalar.dma_start(
                        out=outT[7 * P:8 * P,
                                 MB + h * (MB // 2):MB + (h + 1) * (MB // 2)],
                        in_=ot_sb[3][:, h * (MB // 2):(h + 1) * (MB // 2)],
                    ).then_inc(st_sems[3], 16)
                for j in range(3):
                    scalar.wait_ge(st_sems[j], (NGROUPS // 4) * 16)
                scalar.wait_ge(st_sems[3], 5 * 16)

            @block.tensor
            def _(tensor):
                # Three k-outer phases over 4 PSUM banks each. Group ids
                # (= mm/ev order): P0 -> g0-3 (nt0-3, mb0, banks 0-3),
                # P1 -> g4-7 (nt4-7, mb0, banks 4-7), P2 -> g8-11
                # (nt0-3, mb1, banks 0-3). Each k-slice feeds 4 matmuls
                # as soon as it lands.
                for phase in range(3):
                    mb = phase // 2          # 0,0,1
                    cw = phase % 2           # wt column block 0,1,0
                    bank0 = cw * 4           # banks 0-3 / 4-7
                    if phase == 2:
                        tensor.wait_ge(ev, 4)   # banks 0-3 evicted (P0)
                    for k in range(KT):
                        if phase == 0:
                            tensor.wait_ge(ld_w[k][0], 16)
                            tensor.wait_ge(ld_x[k][0], 16)
                        elif phase == 1:
                            tensor.wait_ge(ld_w[k][1], 16)
                        elif phase == 2:
                            tensor.wait_ge(ld_x[k][1], 16)
                        for j in range(4):
                            inst = tensor.matmul(
                                ps[bank0 + j][:, :],
                                wt_sb[k][cw][:, j * P:(j + 1) * P],
                                xt_sb[k][mb][:, :],
                                start=(k == 0),
                                stop=(k == KT - 1),
                            )
                            if k == KT - 1:
                                inst.then_inc(mm, 1)
                # Last phase (nt4-7, mb1, banks 4-7) k-inner: group
                # completions land ~1.9us apart so evictions + stores
                # pipeline instead of bunching at the end.
                tensor.wait_ge(ev, 8)   # banks 4-7 evicted (P1)
                for g in range(12, NGROUPS):
                    nt = g - 8
                    ni = nt - 4
                    inst = None
                    for k in range(KT):
                        inst = tensor.matmul(
                            ps[4 + ni][:, :],
                            wt_sb[k][1][:, ni * P:(ni + 1) * P],
                            xt_sb[k][1][:, :],
                            start=(k == 0),
                            stop=(k == KT - 1),
                        )
                    inst.then_inc(mm, 1)

            @block.vector
            def _(vector):
                vector.wait_ge(ld_b, 16)
                for g in range(NGROUPS - 1):
                    mb, nt = divmod(g, NT)
                    vector.wait_ge(mm, g + 1)
                    if g >= 4:
                        # ot slot g%4 reused: all issued slot stores
                        # (groups g%4, g%4+4, ..., g-4) must be done
                        vector.wait_ge(st_sems[g % 4], (g // 4) * 16)
                    vector.tensor_scalar_add(
                        ot_sb[g % 4][:],
                        ps[g % 8][:, :],
                        bias_sb[:, nt:nt + 1],
                    ).then_inc(ev, 1)
                # Last group in halves: first half's store overlaps the
                # second half's eviction, shortening the critical tail.
                vector.wait_ge(mm, NGROUPS)
                vector.wait_ge(st_sems[3], 48)
                for h in range(2):
                    vector.tensor_scalar_add(
                        ot_sb[3][:, h * (MB // 2):(h + 1) * (MB // 2)],
                        ps[7][:, h * (MB // 2):(h + 1) * (MB // 2)],
                        bias_sb[:, 7:8],
                    ).then_inc(ev_h, 1)

    return nc


_PROGRAM = None


def _get_program() -> bass.Bass:
    global _PROGRAM
    if _PROGRAM is None:
        _PROGRAM = build_program()
    return _PROGRAM


def make_in_maps(x: np.ndarray, W: np.ndarray, b: np.ndarray) -> list[dict]:
    WT = np.ascontiguousarray(W.T.astype(ml_dtypes.bfloat16))
    bias = np.ascontiguousarray(
        b.astype(np.float32, copy=False).reshape(NT, P).T
    )
    in_maps = []
    for c in range(N_CORES):
        xT = np.ascontiguousarray(x[c * M:(c + 1) * M, :].T.astype(ml_dtypes.bfloat16))
        in_maps.append({"xT": xT, "wT": WT, "bias": bias})
    return in_maps


def assemble_output(results: list[dict]) -> np.ndarray:
    out = np.empty((B, OUT_F), dtype=np.float32)
    for c in range(N_CORES):
        out[c * M:(c + 1) * M, :] = results[c]["outT"].T.astype(np.float32)
    return out


def kernel(x: np.ndarray, W: np.ndarray, b: np.ndarray) -> np.ndarray:
    nc = _get_program()
    in_maps = make_in_maps(np.asarray(x), np.asarray(W), np.asarray(b))
    res = run_bass_kernel_spmd(nc, in_maps, list(range(N_CORES)))
    return assemble_output(res.results)


# revision 10
# speedup vs baseline: 1.1782x; 1.0708x over previous
"""Dense MLP y = x @ W.T + b on 8 TRN2 NeuronCores, data-parallel over batch.

Full inputs: x [8192, 1024] f32, W [1024, 1024] f32, b [1024] f32.
Each core computes a [1024, 1024] slice of the output.

Per-core kernel computes the transposed output
    outT[n, m] = sum_k WT[k, n] * xT[k, m] + b[n]
so the bias lands on the partition dim (n) and fuses into the PSUM
eviction as a DVE tensor_scalar add. Host pre-transposes x-shards and W
to K-major (contraction on partitions) and un-transposes the gathered
outputs; only device time counts.

v4: bf16 end to end (matmul inputs AND the stored output; PSUM still
accumulates f32). Measured rel err ~4e-3 vs the 2e-2 gate. Why bf16:
the v3 trace showed the kernel DMA-bound end to end - 8 MB of f32r
loads + 4 MB of f32 stores at the ~310 GB/s aggregate fabric rate
meant the last x-slice landed at ~40 us of a 54 us kernel. bf16 halves
every DMA byte (4 MB loads + 2 MB stores) while the PE still streams
1 cycle/row at moving dim 512, so the kernel flips to PE-bound:
~27.3 us of matmul + ~7.2 us fixed framework preamble + DMA spin-up.

v5 (trace-driven on top of v4): the v4 trace showed every
DMA_DIRECT2D costs ~650 ns of QUEUE time regardless of transfer size
(descriptor generation: 128 partition-line descriptors x ~5 ns), so
32 load DMAs on one queue = 21 us of issue time - the loads were
issue-bound, not byte-bound. Two changes:
  1. Loads split across TWO HWDGE queues issuing in parallel: sync
     carries the 16 w-block DMAs, vector carries the 16 x-block DMAs.
     Per-queue issue rate ~197 GB/s; the two together match the
     ~390 GB/s fabric cap, and P0's gate (w_c0 + x_c0, 2 MB) lands in
     half the time.
  2. 16 warmup matmuls (128 rows each, on a memset tile, into ps[7]
     which the first start=True real matmul later resets) issued
     before the first data-dependent matmul: the PE clock ramps to
     full speed (~3 us of sustained activity) while the first loads
     are still in flight, instead of burning the ramp on real work.

Raw Bass (no TileContext: its exit drain trips "Too many sync wait
commands" in this compiler build).

Engine layout (trace-driven):
  sync:   w-block load DMAs (w_c0 k0-7, then w_c1 k0-7).
  vector: x-block load DMAs (x_c0 k0-7, then x_c1 k0-7), then the
          PSUM->SBUF evictions with fused bias add (f32 psum -> bf16).
  scalar: output stores (idle queue; stores only need a trickle and
          must not displace load descriptors).
  gpsimd: warmup-tile memset, then bias load (SWDGE, off the critical
          queues).
  tensor: warmup, then four k-outer phases over 4 PSUM banks each with
          per-slice gating - each k-slice feeds 4 matmuls the moment
          it lands.
"""

import numpy as np
import ml_dtypes

import concourse.bass as bass
import concourse.mybir as mybir
from concourse.bass_utils import run_bass_kernel_spmd

B, IN_F, OUT_F = 8192, 1024, 1024
N_CORES = 8
M = B // N_CORES  # batch rows per core
P = 128           # partitions
MB = 512          # moving-dim block (one PSUM bank of fp32)
KT = IN_F // P    # k tiles (8)
NT = OUT_F // P   # n tiles (8)
CB = 512          # column-block width (1KB bf16 DMA lines per partition)
NGROUPS = (M // MB) * NT  # 16 psum groups, order g = mb*NT + nt

F32 = mybir.dt.float32
BF16 = mybir.dt.bfloat16


def build_program() -> bass.Bass:
    nc = bass.Bass()
    xT = nc.declare_dram_parameter("xT", [IN_F, M], BF16, isOutput=False)
    wT = nc.declare_dram_parameter("wT", [IN_F, OUT_F], BF16, isOutput=False)
    bias = nc.declare_dram_parameter("bias", [P, NT], F32, isOutput=False)
    outT = nc.declare_dram_parameter("outT", [OUT_F, M], BF16, isOutput=True)

    import contextlib

    with contextlib.ExitStack() as ctx:
        wt_sb = [
            [ctx.enter_context(nc.sbuf_tensor(f"wt{k}_{c}", [P, CB], BF16))
             for c in range(2)]
            for k in range(KT)
        ]
        xt_sb = [
            [ctx.enter_context(nc.sbuf_tensor(f"xt{k}_{c}", [P, CB], BF16))
             for c in range(2)]
            for k in range(KT)
        ]
        ot_sb = [
            ctx.enter_context(nc.sbuf_tensor(f"ot{j}", [P, MB], BF16))
            for j in range(4)
        ]
        bias_sb = ctx.enter_context(nc.sbuf_tensor("bias_sb", [P, NT], F32))
        warm_sb = ctx.enter_context(nc.sbuf_tensor("warm_sb", [P, P], BF16))
        ps = [
            ctx.enter_context(nc.psum_tensor(f"ps{b}", [P, MB], F32))
            for b in range(8)
        ]
        ld_b = ctx.enter_context(nc.semaphore("ld_b"))
        # Per (k-slice, column-block) load sems: a shared counter can't
        # prove a *specific* DMA finished (completions are unordered),
        # a single-incrementer sem can.
        ld_w = [
            [ctx.enter_context(nc.semaphore(f"ld_w{k}_{c}")) for c in range(2)]
            for k in range(KT)
        ]
        ld_x = [
            [ctx.enter_context(nc.semaphore(f"ld_x{k}_{c}")) for c in range(2)]
            for k in range(KT)
        ]
        warm = ctx.enter_context(nc.semaphore("warm"))
        mm = ctx.enter_context(nc.semaphore("mm"))
        ev = ctx.enter_context(nc.semaphore("ev"))
        ev_h = ctx.enter_context(nc.semaphore("ev_h"))  # last-group halves
        # Per-ot-slot store sems (same unordered-completion argument).
        st_sems = [
            ctx.enter_context(nc.semaphore(f"st{j}")) for j in range(4)
        ]

        with nc.Block(no_gpsimd_drain=True) as block:

            @block.sync
            def _(sync):
                # w loads only: DMA issue costs ~650 ns per instruction
                # (descriptor gen), so w and x each get their own queue
                # and issue in parallel at ~197 GB/s apiece.
                for c in range(2):
                    for k in range(KT):
                        sync.dma_start(
                            out=wt_sb[k][c][:],
                            in_=wT[k * P:(k + 1) * P, c * CB:(c + 1) * CB],
                        ).then_inc(ld_w[k][c], 16)

            @block.gpsimd
            def _(gpsimd):
                gpsimd.memset(warm_sb[:], 0).then_inc(warm, 1)
                gpsimd.dma_start(out=bias_sb[:], in_=bias[:]).then_inc(ld_b, 16)

            @block.scalar
            def _(scalar):
                # x loads first: issue in parallel with sync's w loads
                # (HWDGE queues are only SP + Activation; DVE can't
                # trigger DMAs). All 16 x issues are done ~2 us before
                # the first store is needed, and the w_c1 tail only
                # overlaps the first store transfers briefly.
                for c in range(2):
                    for k in range(KT):
                        scalar.dma_start(
                            out=xt_sb[k][c][:],
                            in_=xT[k * P:(k + 1) * P, c * CB:(c + 1) * CB],
                        ).then_inc(ld_x[k][c], 16)
                # Stores follow on the same queue. Last group is split
                # in half so the final eviction->store->drain chain is
                # shorter.
                for g in range(NGROUPS - 1):
                    mb, nt = divmod(g, NT)
                    scalar.wait_ge(ev, g + 1)
                    scalar.dma_start(
                        out=outT[nt * P:(nt + 1) * P, mb * MB:(mb + 1) * MB],
                        in_=ot_sb[g % 4][:],
                    ).then_inc(st_sems[g % 4], 16)
                for h in range(2):
                    scalar.wait_ge(ev_h, h + 1)
                    scalar.dma_start(
                        out=outT[7 * P:8 * P,
                                 MB + h * (MB // 2):MB + (h + 1) * (MB // 2)],
                        in_=ot_sb[3][:, h * (MB // 2):(h + 1) * (MB // 2)],
                    ).then_inc(st_sems[3], 16)
                for j in range(3):
                    scalar.wait_ge(st_sems[j], (NGROUPS // 4) * 16)
                scalar.wait_ge(st_sems[3], 5 * 16)

            @block.tensor
            def _(tensor):
                # Warmup: 16 x 128-row matmuls on the memset tile while
                # the first loads are in flight. The PE clock needs
                # ~3 us of sustained activity to reach full speed;
                # burning the ramp on zeros means the first real
                # matmuls stream at full rate. ps[7] is reset by P1's
                # start=True before any real accumulation.
                tensor.wait_ge(warm, 1)
                for _ in range(16):
                    tensor.matmul(
                        ps[7][:, 0:P],
                        warm_sb[:, :],
                        warm_sb[:, :],
                        start=True,
                        stop=True,
                    )
                # Three k-outer phases over 4 PSUM banks each. Group ids
                # (= mm/ev order): P0 -> g0-3 (nt0-3, mb0, banks 0-3),
                # P1 -> g4-7 (nt4-7, mb0, banks 4-7), P2 -> g8-11
                # (nt0-3, mb1, banks 0-3). Each k-slice feeds 4 matmuls
                # as soon as it lands.
                for phase in range(3):
                    mb = phase // 2          # 0,0,1
                    cw = phase % 2           # wt column block 0,1,0
                    bank0 = cw * 4           # banks 0-3 / 4-7
                    if phase == 2:
                        tensor.wait_ge(ev, 4)   # banks 0-3 evicted (P0)
                    for k in range(KT):
                        if phase == 0:
                            tensor.wait_ge(ld_w[k][0], 16)
                            tensor.wait_ge(ld_x[k][0], 16)
                        elif phase == 1:
                            tensor.wait_ge(ld_w[k][1], 16)
                        elif phase == 2:
                            tensor.wait_ge(ld_x[k][1], 16)
                        for j in range(4):
                            inst = tensor.matmul(
                                ps[bank0 + j][:, :],
                                wt_sb[k][cw][:, j * P:(j + 1) * P],
                                xt_sb[k][mb][:, :],
                                start=(k == 0),
                                stop=(k == KT - 1),
                            )
                            if k == KT - 1:
                                inst.then_inc(mm, 1)
                # Last phase (nt4-7, mb1, banks 4-7) k-inner: group
                # completions land ~1.9us apart so evictions + stores
                # pipeline instead of bunching at the end.
                tensor.wait_ge(ev, 8)   # banks 4-7 evicted (P1)
                for g in range(12, NGROUPS):
                    nt = g - 8
                    ni = nt - 4
                    inst = None
                    for k in range(KT):
                        inst = tensor.matmul(
                            ps[4 + ni][:, :],
                            wt_sb[k][1][:, ni * P:(ni + 1) * P],
                            xt_sb[k][1][:, :],
                            start=(k == 0),
                            stop=(k == KT - 1),
                        )
                    inst.then_inc(mm, 1)

            @block.vector
            def _(vector):
                vector.wait_ge(ld_b, 16)
                for g in range(NGROUPS - 1):
                    mb, nt = divmod(g, NT)
                    vector.wait_ge(mm, g + 1)
                    if g >= 4:
                        # ot slot g%4 reused: all issued slot stores
                        # (groups g%4, g%4+4, ..., g-4) must be done
                        vector.wait_ge(st_sems[g % 4], (g // 4) * 16)
                    vector.tensor_scalar_add(
                        ot_sb[g % 4][:],
                        ps[g % 8][:, :],
                        bias_sb[:, nt:nt + 1],
                    ).then_inc(ev, 1)
                # Last group in halves: first half's store overlaps the
                # second half's eviction, shortening the critical tail.
                vector.wait_ge(mm, NGROUPS)
                vector.wait_ge(st_sems[3], 48)
                for h in range(2):
                    vector.tensor_scalar_add(
                        ot_sb[3][:, h * (MB // 2):(h + 1) * (MB // 2)],
                        ps[7][:, h * (MB // 2):(h + 1) * (MB // 2)],
                        bias_sb[:, 7:8],
                    ).then_inc(ev_h, 1)

    return nc


_PROGRAM = None


def _get_program() -> bass.Bass:
    global _PROGRAM
    if _PROGRAM is None:
        _PROGRAM = build_program()
    return _PROGRAM


def make_in_maps(x: np.ndarray, W: np.ndarray, b: np.ndarray) -> list[dict]:
    WT = np.ascontiguousarray(W.T.astype(ml_dtypes.bfloat16))
    bias = np.ascontiguousarray(
        b.astype(np.float32, copy=False).reshape(NT, P).T
    )
    in_maps = []
    for c in range(N_CORES):
        xT = np.ascontiguousarray(x[c * M:(c + 1) * M, :].T.astype(ml_dtypes.bfloat16))
        in_maps.append({"xT": xT, "wT": WT, "bias": bias})
    return in_maps


def assemble_output(results: list[dict]) -> np.ndarray:
    out = np.empty((B, OUT_F), dtype=np.float32)
    for c in range(N_CORES):
        out[c * M:(c + 1) * M, :] = results[c]["outT"].T.astype(np.float32)
    return out


def kernel(x: np.ndarray, W: np.ndarray, b: np.ndarray) -> np.ndarray:
    nc = _get_program()
    in_maps = make_in_maps(np.asarray(x), np.asarray(W), np.asarray(b))
    res = run_bass_kernel_spmd(nc, in_maps, list(range(N_CORES)))
    return assemble_output(res.results)
